# revision 2
# baseline (speedup 1.0000x reference)
"""BiGRU Trainium2 kernel (Bass/Tile), SPMD over 8 NeuronCores.

Sharding: one GRU direction per core (cores 0-3 fwd, 4-7 bwd; same program,
different data), 32 batch rows per core. Each core emits the partial
fc dot-product for its direction; host adds fwd+bwd partials and applies
the final sigmoid (128 scalar ops).

Layout ("gate-as-M"): hidden state and all gate tensors live as
[H-on-partitions, batch-on-free] tiles ([128, 4, 32] per 512-channel
quantity). The scan matmul makes the *weights* the stationary operand
(lhsT = Whh.T chunk [128,128]) and h the moving operand ([128,32]).

Critical-path schedule (per step):
  PE:  32 r/z matmuls -> (sig) -> 17 n matmuls -> xp-share for next block
  ACT: sigmoid(r,z) emitted RIGHT AFTER the r/z matmuls so its PE
       watermark excludes the n matmuls; tanh after v.
  DVE: t2 = r*nh; v = t2 + xp_n; a = z'*n; h' = a + b2 (in-queue).
  GPSIMD: zf = 1-z', b2 = zf*h right after sig (ready long before h').

x-projection (xp) for the NEXT 4-step block is *spread* across the four
steps' PE tails (bias chunk + 3 wih chunks per step) so it executes in
PE idle time during the activation chain instead of stalling the first
step of each block. This also keeps the PE HAM-warm. The n-gate bias is
hoisted to one [128, 4*128] indicator matmul per block (nh4 psum).

z is computed as z' = 1-z = sigmoid(-(...)) by pre-negating z weights/bias
on the host; h' = z'*n + (1-z')*h with (1-z') and (1-z')*h computed
off-critical-path (hidden under tanh).
"""

import numpy as np
import ml_dtypes

import concourse.bass as bass
import concourse.bacc as bacc
import concourse.mybir as mybir
from concourse import tile
from concourse.bass_utils import run_bass_kernel_spmd

BF = ml_dtypes.bfloat16
V, E, H = 50000, 256, 512
B, T = 128, 512
NC = 8
BL = 32               # batch rows per core (one direction per core)
G = 3 * H             # 1536
EK = E // 128         # 2
HK = H // 128         # 4
NG = G // 128         # 12 gate chunks: r 0-3, z 4-7, n 8-11
TP = T + 8            # padded steps so the xp pipeline can run ahead
U = 32                # steps unrolled per hardware-loop iteration

bf = mybir.dt.bfloat16
f32 = mybir.dt.float32
ACT = mybir.ActivationFunctionType
ALU = mybir.AluOpType

# xp spread plan: per step s of a block, which bias group / wih chunks to emit
XP_BIAS = {0: 0, 1: 1, 2: 2}            # step -> bias group (g covers chunks 4g..4g+3)
XP_CHUNKS = {0: (0, 1, 2), 1: (3, 4, 5), 2: (6, 7, 8), 3: (9, 10, 11)}


def _build_nc():
    nc = bacc.Bacc(None, target_bir_lowering=False)

    xT = nc.dram_tensor("xT", [EK, 128, TP * BL], bf, kind="ExternalInput")
    whh = nc.dram_tensor("whh", [HK, 128, G], bf, kind="ExternalInput")
    wih = nc.dram_tensor("wih", [EK, 128, G], bf, kind="ExternalInput")
    biasx = nc.dram_tensor("biasx", [NG, 128], bf, kind="ExternalInput")
    bhn = nc.dram_tensor("bhn", [HK, 128], bf, kind="ExternalInput")
    indx = nc.dram_tensor("indx", [NG, NG * 128], bf, kind="ExternalInput")
    indh = nc.dram_tensor("indh", [HK, 4 * HK * BL], bf, kind="ExternalInput")
    fcw = nc.dram_tensor("fcw", [128, HK], bf, kind="ExternalInput")
    ones = nc.dram_tensor("ones", [1, BL], bf, kind="ExternalInput")
    yout = nc.dram_tensor("y", [1, BL], f32, kind="ExternalOutput")

    with tile.TileContext(nc) as tc:
        with (
            tc.tile_pool(name="cst", bufs=1) as cst,
            tc.tile_pool(name="wk", bufs=3) as wk,
            tc.tile_pool(name="ps", bufs=2, space="PSUM") as ps,
        ):
            # ---- resident SBUF constants ----
            xT_sb = cst.tile([128, EK, TP * BL], bf, tag="xT")
            for e in range(EK):
                nc.sync.dma_start(xT_sb[:, e, :], xT[e])
            whh_sb = cst.tile([128, HK, G], bf, tag="whh")
            for k in range(HK):
                nc.sync.dma_start(whh_sb[:, k, :], whh[k])
            wih_sb = cst.tile([128, EK, G], bf, tag="wih")
            for e in range(EK):
                nc.sync.dma_start(wih_sb[:, e, :], wih[e])
            biasx_sb = cst.tile([NG, 128], bf, tag="biasx")
            nc.sync.dma_start(biasx_sb[:, :], biasx[:, :])
            bhn_sb = cst.tile([HK, 128], bf, tag="bhn")
            nc.sync.dma_start(bhn_sb[:, :], bhn[:, :])
            indx_sb = cst.tile([NG, NG * 128], bf, tag="indx")
            nc.sync.dma_start(indx_sb[:, :], indx[:, :])
            indh_sb = cst.tile([HK, 4 * HK * BL], bf, tag="indh")
            nc.sync.dma_start(indh_sb[:, :], indh[:, :])
            fcw_sb = cst.tile([128, HK], bf, tag="fcw")
            nc.sync.dma_start(fcw_sb[:, :], fcw[:, :])
            ones_sb = cst.tile([1, BL], bf, tag="ones")
            nc.sync.dma_start(ones_sb[:, :], ones[:, :])

            # hidden state, ping-ponged per step: [128, HK, BL] bf16
            h_sb = [cst.tile([128, HK, BL], bf, tag=f"h{i}", name=f"h{i}")
                    for i in range(4)]
            nc.vector.memzero(h_sb[0][:, :, :])

            # ---- warmup: absorb each input DMA's completion wait into its
            # own trivial PE matmul (in-loop matmuls must not carry >1
            # outstanding dependency: the Ldweights uinstruction has a
            # single sync-wait slot) ----
            warm_ps = ps.tile([128, NG * 128], f32, tag="xpA", name="warm")
            touches = (
                [xT_sb[0:1, e, 0:BL] for e in range(EK)]
                + [whh_sb[0:1, k, 0:BL] for k in range(HK)]
                + [wih_sb[0:1, e, 0:BL] for e in range(EK)]
                + [biasx_sb[0:1, 0:BL], bhn_sb[0:1, 0:BL],
                   indx_sb[0:1, 0:BL], indh_sb[0:1, 0:BL],
                   fcw_sb[0:1, 0:HK]]
            )
            first = True
            for src in touches:
                nc.tensor.matmul(warm_ps[0:1, 0:src.free_size()],
                                 ones_sb[:, 0:1], src,
                                 start=first, stop=False)
                first = False
            nc.tensor.matmul(warm_ps[0:1, 0:1], ones_sb[:, 0:1],
                             ones_sb[:, 0:1], start=False, stop=True)
            scrap = cst.tile([1, BL], bf, tag="scrap")
            nc.gpsimd.tensor_copy(scrap[0:1, :], xT_sb[0:1, 0, 0:BL])
            nc.scalar.activation(scrap[0:1, :], scrap[0:1, :], ACT.Sigmoid)
            nc.vector.tensor_copy(scrap[0:1, :], xT_sb[0:1, 1, 0:BL])

            # ---- xp production for a 4-step block ----
            def xp_stage(t0):
                """Stage x for steps [t0, t0+4) through a static SBUF buffer
                (GPSIMD copy) so in-loop PE reads are register-free."""
                xs = wk.tile([128, EK, 4 * BL], bf, tag="xs", name="xs")
                for e in range(EK):
                    nc.gpsimd.tensor_copy(xs[:, e, :],
                                          xT_sb[:, e, bass.ds(t0 * BL, 4 * BL)])
                return xs

            def xp_mms(blk, xs, s):
                """Emit this step's share of the next block's xp matmuls."""
                if s in XP_BIAS:
                    g = XP_BIAS[s]
                    nc.tensor.matmul(blk[:, g * 512:(g + 1) * 512],
                                     biasx_sb[:, :],
                                     indx_sb[:, g * 512:(g + 1) * 512],
                                     start=True, stop=False)
                for c in XP_CHUNKS[s]:
                    for e in range(EK):
                        nc.tensor.matmul(
                            blk[:, c * 128:(c + 1) * 128],
                            wih_sb[:, e, c * 128:(c + 1) * 128],
                            xs[:, e, :],
                            start=False, stop=(e == EK - 1),
                            skip_group_check=True)

            def nh4_bias(nh4):
                """One indicator matmul fills the block's n-gate bias for all
                4 steps: nh4[p, s, c*BL+b] = bhn[c*128+p]."""
                nc.tensor.matmul(nh4[:, :], bhn_sb[:, :], indh_sb[:, :],
                                 start=True, stop=False)

            def step(blk, nh4, s, h_cur, h_nxt, tail):
                """One GRU step: consumes column s (0..3) of psum block."""
                co = s * BL  # column offset of this step inside the block

                # r/z h-projection accumulates straight onto xp+bias
                for c in range(8):
                    for k in range(HK):
                        nc.tensor.matmul(
                            blk[:, c * 128 + co: c * 128 + co + BL],
                            whh_sb[:, k, c * 128:(c + 1) * 128],
                            h_cur[:, k, :],
                            start=False, stop=(k == HK - 1),
                            skip_group_check=True)

                # fused sigmoid over all r,z chunks -> bf16. Emitted BEFORE
                # the n matmuls: its PE watermark covers only the r/z ones.
                rz = wk.tile([128, 8, BL], bf, tag="rz", name="rz")
                nc.scalar.activation(
                    rz[:, :, :],
                    blk[:, :].rearrange("p (c n) -> p c n", c=NG)[:, 0:8, co:co + BL],
                    ACT.Sigmoid)
                # off-chain: z = 1 - z'; b2 = z*h   (ready well before h')
                # (GPSIMD/Pool has no PSUM access: all PSUM readers go to
                # DVE or ACT; all-SBUF bf16 ops go to Pool/DVE-4x.)
                zf = wk.tile([128, HK, BL], bf, tag="zf", name="zf")
                nc.gpsimd.tensor_scalar(zf[:, :, :], rz[:, 4:8, :], -1.0, 1.0,
                                        ALU.mult, ALU.add)
                b2 = wk.tile([128, HK, BL], bf, tag="b2", name="b2")
                nc.gpsimd.tensor_mul(b2[:, :, :], zf[:, :, :], h_cur[:, :, :])

                # n-gate h-projection accumulates onto the block nh4 bias
                for c in range(HK):
                    for k in range(HK):
                        nc.tensor.matmul(
                            nh4[:, s, c * BL:(c + 1) * BL],
                            whh_sb[:, k, (8 + c) * 128:(9 + c) * 128],
                            h_cur[:, k, :],
                            start=False, stop=(k == HK - 1),
                            skip_group_check=True)

                # n chain: t2 = r * (hn + bhn); v = t2 + xp_n; n = tanh(v)
                t2 = wk.tile([128, HK, BL], bf, tag="t2", name="t2")
                nc.vector.tensor_mul(
                    t2[:, :, :], rz[:, 0:4, :],
                    nh4[:, s, :].rearrange("p (c n) -> p c n", c=HK))
                v = wk.tile([128, HK, BL], bf, tag="v", name="v")
                nc.vector.tensor_add(
                    v[:, :, :], t2[:, :, :],
                    blk[:, :].rearrange("p (c n) -> p c n", c=NG)[:, 8:12, co:co + BL])
                n_t = wk.tile([128, HK, BL], bf, tag="n", name="n")
                nc.scalar.activation(n_t[:, :, :], v[:, :, :], ACT.Tanh)
                # h' = z'*n + z*h
                a_t = wk.tile([128, HK, BL], bf, tag="a", name="a")
                nc.vector.tensor_mul(a_t[:, :, :], rz[:, 4:8, :], n_t[:, :, :])
                nc.vector.tensor_add(h_nxt[:, :, :], a_t[:, :, :], b2[:, :, :])

                # next block's xp share fills the PE tail of this step
                tail(s)

            # prologue: produce block 0 (steps 0..3) and its nh4 bias
            xs0 = xp_stage(0)
            blk_cur = ps.tile([128, NG * 128], f32, tag="xpA", name="xp")
            for s in range(4):
                xp_mms(blk_cur, xs0, s)
            nh4_cur = ps.tile([128, 4, HK * BL], f32, tag="nh", name="nh4")
            nh4_bias(nh4_cur[:, :, :].rearrange("p c n -> p (c n)"))

            ASSUMED_EVEN_BLOCKS = (U // 4) % 2 == 0
            assert ASSUMED_EVEN_BLOCKS and U % 4 == 0

            with tc.For_i(0, T // U, 1, staggered_reset=True,
                          hint_engines=(mybir.EngineType.PE,)) as it:
                t_base = it * U
                for u4 in range(U // 4):
                    xs_nxt = xp_stage(t_base + u4 * 4 + 4)
                    blk_nxt = ps.tile([128, NG * 128], f32, tag="xpA", name="xp")
                    nh4_nxt = ps.tile([128, 4, HK * BL], f32, tag="nh", name="nh4")

                    def tail(s, b=blk_nxt, x=xs_nxt, n4=nh4_nxt):
                        xp_mms(b, x, s)
                        if s == 3:
                            nh4_bias(n4[:, :, :].rearrange("p c n -> p (c n)"))

                    for s in range(4):
                        t = u4 * 4 + s          # static step index in body
                        step(blk_cur, nh4_cur, s,
                             h_sb[t % 4], h_sb[(t + 1) % 4], tail)
                    blk_cur, nh4_cur = blk_nxt, nh4_nxt

            # ---- fc partial: y[b] = sum_k fcw[:,k] . h[:,k,b] ----
            fc_ps = ps.tile([1, BL], f32, tag="nh", name="fc")
            for k in range(HK):
                nc.tensor.matmul(fc_ps[:, :], fcw_sb[:, k:k + 1],
                                 h_sb[T % 4][:, k, :],
                                 start=(k == 0), stop=(k == HK - 1))
            y_sb = cst.tile([1, BL], f32, tag="y")
            nc.scalar.copy(y_sb[:, :], fc_ps[:, :])
            nc.sync.dma_start(yout[:, :], y_sb[:, :])
    nc.finalize()
    return nc


_NC_CACHE = None


def _get_nc():
    global _NC_CACHE
    if _NC_CACHE is None:
        _NC_CACHE = _build_nc()
    return _NC_CACHE


def _prep_xT(x_c, rev):
    """x_c [BL, T, E] f32 -> [EK, 128, TP*BL] bf16 (optionally reversed)."""
    if rev:
        x_c = x_c[:, ::-1, :]
    xt = np.zeros((EK, 128, TP * BL), np.float32)
    # xt[e, p, t*BL + b] = x_c[b, t, 128e + p]
    xt[:, :, :T * BL] = np.ascontiguousarray(
        x_c.transpose(2, 1, 0)).reshape(EK, 128, T * BL)
    return xt.astype(BF)


def _prep_weights(W_ih, W_hh, b_ih, b_hh):
    Wi = np.array(W_ih, np.float32).copy()
    Wh = np.array(W_hh, np.float32).copy()
    Wi[H:2 * H] *= -1.0
    Wh[H:2 * H] *= -1.0
    # whh[k] = Whh'[:, 128k:128k+128].T  -> [128, G]
    whhT = np.ascontiguousarray(Wh.T).reshape(HK, 128, G).astype(BF)
    wihT = np.ascontiguousarray(Wi.T).reshape(EK, 128, G).astype(BF)
    bi = np.asarray(b_ih, np.float32)
    bh = np.asarray(b_hh, np.float32)
    vb = np.concatenate([
        bi[0:H] + bh[0:H],
        -(bi[H:2 * H] + bh[H:2 * H]),
        bi[2 * H:3 * H]])
    biasx = vb.reshape(NG, 128).astype(BF)
    bhn_ = bh[2 * H:3 * H].reshape(HK, 128).astype(BF)
    return wihT, whhT, biasx, bhn_


def prepare_in_maps(inputs, emb, W_ih_f, W_hh_f, b_ih_f, b_hh_f,
                    W_ih_b, W_hh_b, b_ih_b, b_hh_b, fc_w, fc_b):
    ids = np.asarray(inputs)
    emb = np.asarray(emb, np.float32)
    x = emb[ids]  # [B, T, E]

    indx_ = np.zeros((NG, NG * 128), np.float32)
    for k in range(NG):
        indx_[k, k * 128:(k + 1) * 128] = 1.0
    indh1 = np.zeros((HK, HK * BL), np.float32)
    for k in range(HK):
        indh1[k, k * BL:(k + 1) * BL] = 1.0
    indh_ = np.tile(indh1, (1, 4))          # [HK, 4*HK*BL]
    fc = np.asarray(fc_w, np.float32)[0]
    ones = np.ones((1, BL), np.float32)

    per_dir = {}
    for d, (Wi, Wh, bi, bh) in (
            ("f", (W_ih_f, W_hh_f, b_ih_f, b_hh_f)),
            ("b", (W_ih_b, W_hh_b, b_ih_b, b_hh_b))):
        wihT, whhT, biasx, bhn_ = _prep_weights(Wi, Wh, bi, bh)
        off = 0 if d == "f" else H
        fcw_ = fc[off:off + H].reshape(HK, 128).T.copy().astype(BF)  # [128, HK]
        per_dir[d] = dict(whh=whhT, wih=wihT, biasx=biasx, bhn=bhn_,
                          fcw=fcw_, indx=indx_.astype(BF),
                          indh=indh_.astype(BF), ones=ones.astype(BF))

    in_maps = []
    for c in range(NC):
        d = "f" if c < 4 else "b"
        sh = c % 4
        x_c = x[sh * BL:(sh + 1) * BL]
        in_maps.append(dict(per_dir[d], xT=_prep_xT(x_c, d == "b")))
    return in_maps


def kernel(**inputs):
    in_maps = prepare_in_maps(**inputs)
    nc = _get_nc()
    res = run_bass_kernel_spmd(nc, in_maps, core_ids=list(range(NC)))
    fcb = np.float32(np.asarray(inputs["fc_b"], np.float32).reshape(-1)[0])
    out = np.empty((B, 1), np.float32)
    for sh in range(4):
        yf = np.asarray(res.results[sh]["y"], np.float32).reshape(BL)
        yb = np.asarray(res.results[4 + sh]["y"], np.float32).reshape(BL)
        out[sh * BL:(sh + 1) * BL, 0] = 1.0 / (1.0 + np.exp(-(yf + yb + fcb)))
    return out


# revision 8
# speedup vs baseline: 1.0409x; 1.0409x over previous
"""BiGRU Trainium2 kernel (Bass/Tile), SPMD over 8 NeuronCores.

Sharding: one GRU direction per core (cores 0-3 fwd, 4-7 bwd; same program,
different data), 32 batch rows per core. Each core emits the partial
fc dot-product for its direction; host adds fwd+bwd partials and applies
the final sigmoid (128 scalar ops).

Layout ("gate-as-M"): hidden state and all gate tensors live as
[H-on-partitions, batch-on-free] tiles ([128, 4, 32] per 512-channel
quantity). The scan matmul makes the *weights* the stationary operand
(lhsT = Whh.T chunk [128,128]) and h the moving operand ([128,32]).

Critical-path schedule (per step):
  PE:  32 r/z matmuls -> (sig) -> 17 n matmuls -> xp-share for next block
  ACT: sigmoid(r,z) emitted RIGHT AFTER the r/z matmuls so its PE
       watermark excludes the n matmuls; tanh after v.
  DVE: t2 = r*nh; v = t2 + xp_n; a = z'*n; h' = a + b2 (in-queue).
  GPSIMD: zf = 1-z', b2 = zf*h right after sig (ready long before h').

x-projection (xp) for the NEXT 4-step block is *spread* across the four
steps' PE tails (bias chunk + 3 wih chunks per step) so it executes in
PE idle time during the activation chain instead of stalling the first
step of each block. This also keeps the PE HAM-warm. The n-gate bias is
hoisted to one [128, 4*128] indicator matmul per block (nh4 psum).

z is computed as z' = 1-z = sigmoid(-(...)) by pre-negating z weights/bias
on the host; h' = z'*n + (1-z')*h with (1-z') and (1-z')*h computed
off-critical-path (hidden under tanh).
"""

import numpy as np
import ml_dtypes

import concourse.bass as bass
import concourse.bacc as bacc
import concourse.mybir as mybir
from concourse import tile
from concourse.bass_utils import run_bass_kernel_spmd

BF = ml_dtypes.bfloat16
V, E, H = 50000, 256, 512
B, T = 128, 512
NC = 8
BL = 32               # batch rows per core (one direction per core)
G = 3 * H             # 1536
EK = E // 128         # 2
HK = H // 128         # 4
NG = G // 128         # 12 gate chunks: r 0-3, z 4-7, n 8-11
TP = T + 8            # padded steps so the xp pipeline can run ahead
U = 32                # steps unrolled per hardware-loop iteration

bf = mybir.dt.bfloat16
f32 = mybir.dt.float32
ACT = mybir.ActivationFunctionType
ALU = mybir.AluOpType

# xp spread plan: per step s of a block, which bias group / wih chunks to emit
XP_BIAS = {0: 0, 1: 1, 2: 2}            # step -> bias group (g covers chunks 4g..4g+3)
XP_CHUNKS = {0: (0, 1, 2), 1: (3, 4, 5), 2: (6, 7, 8), 3: (9, 10, 11)}


def _build_nc():
    nc = bacc.Bacc(None, target_bir_lowering=False)

    xT = nc.dram_tensor("xT", [EK, 128, TP * BL], bf, kind="ExternalInput")
    whh = nc.dram_tensor("whh", [HK, 128, G], bf, kind="ExternalInput")
    wih = nc.dram_tensor("wih", [EK, 128, G], bf, kind="ExternalInput")
    biasx = nc.dram_tensor("biasx", [NG, 128], bf, kind="ExternalInput")
    bhn = nc.dram_tensor("bhn", [HK, 128], bf, kind="ExternalInput")
    indx = nc.dram_tensor("indx", [NG, NG * 128], bf, kind="ExternalInput")
    indh = nc.dram_tensor("indh", [HK, 4 * HK * BL], bf, kind="ExternalInput")
    fcw = nc.dram_tensor("fcw", [128, HK], bf, kind="ExternalInput")
    ones = nc.dram_tensor("ones", [1, BL], bf, kind="ExternalInput")
    yout = nc.dram_tensor("y", [1, BL], f32, kind="ExternalOutput")

    with tile.TileContext(nc) as tc:
        with (
            tc.tile_pool(name="cst", bufs=1) as cst,
            tc.tile_pool(name="wk", bufs=3) as wk,
            tc.tile_pool(name="ps", bufs=2, space="PSUM") as ps,
        ):
            # ---- resident SBUF constants ----
            xT_sb = cst.tile([128, EK, TP * BL], bf, tag="xT")
            for e in range(EK):
                nc.sync.dma_start(xT_sb[:, e, :], xT[e])
            whh_sb = cst.tile([128, HK, G], bf, tag="whh")
            for k in range(HK):
                nc.sync.dma_start(whh_sb[:, k, :], whh[k])
            wih_sb = cst.tile([128, EK, G], bf, tag="wih")
            for e in range(EK):
                nc.sync.dma_start(wih_sb[:, e, :], wih[e])
            biasx_sb = cst.tile([NG, 128], bf, tag="biasx")
            nc.sync.dma_start(biasx_sb[:, :], biasx[:, :])
            bhn_sb = cst.tile([HK, 128], bf, tag="bhn")
            nc.sync.dma_start(bhn_sb[:, :], bhn[:, :])
            indx_sb = cst.tile([NG, NG * 128], bf, tag="indx")
            nc.sync.dma_start(indx_sb[:, :], indx[:, :])
            indh_sb = cst.tile([HK, 4 * HK * BL], bf, tag="indh")
            nc.sync.dma_start(indh_sb[:, :], indh[:, :])
            fcw_sb = cst.tile([128, HK], bf, tag="fcw")
            nc.sync.dma_start(fcw_sb[:, :], fcw[:, :])
            ones_sb = cst.tile([1, BL], bf, tag="ones")
            nc.sync.dma_start(ones_sb[:, :], ones[:, :])

            # hidden state, ping-ponged per step: [128, HK, BL] bf16
            h_sb = [cst.tile([128, HK, BL], bf, tag=f"h{i}", name=f"h{i}")
                    for i in range(4)]
            nc.vector.memzero(h_sb[0][:, :, :])

            # ---- static PSUM double-buffers (allocated once; manual
            # ping-pong). Per-group pool-slot rotation made every group's
            # xp writes wait on the slot's previous occupant, which
            # serialized the spread-out xp tails behind the whole group.
            warm_ps = ps.tile([128, NG * 128], f32, tag="xpA", name="warm")
            blk_bufs = [warm_ps if i == 0 else
                        ps.tile([128, NG * 128], f32, tag="xpA", name=f"xpb{i}")
                        for i in range(2)]
            nh_bufs = [ps.tile([128, 4, HK * BL], f32, tag="nh", name=f"nh4b{i}")
                       for i in range(2)]
            touches = (
                [xT_sb[0:1, e, 0:BL] for e in range(EK)]
                + [whh_sb[0:1, k, 0:BL] for k in range(HK)]
                + [wih_sb[0:1, e, 0:BL] for e in range(EK)]
                + [biasx_sb[0:1, 0:BL], bhn_sb[0:1, 0:BL],
                   indx_sb[0:1, 0:BL], indh_sb[0:1, 0:BL],
                   fcw_sb[0:1, 0:HK]]
            )
            first = True
            for src in touches:
                nc.tensor.matmul(warm_ps[0:1, 0:src.free_size()],
                                 ones_sb[:, 0:1], src,
                                 start=first, stop=False)
                first = False
            nc.tensor.matmul(warm_ps[0:1, 0:1], ones_sb[:, 0:1],
                             ones_sb[:, 0:1], start=False, stop=True)
            scrap = cst.tile([1, BL], bf, tag="scrap")
            nc.gpsimd.tensor_copy(scrap[0:1, :], xT_sb[0:1, 0, 0:BL])
            nc.scalar.activation(scrap[0:1, :], scrap[0:1, :], ACT.Sigmoid)
            nc.vector.tensor_copy(scrap[0:1, :], xT_sb[0:1, 1, 0:BL])

            # ---- xp production for a 4-step block ----
            def xp_stage(t0):
                """Stage x for steps [t0, t0+4) through a static SBUF buffer
                (GPSIMD copy) so in-loop PE reads are register-free."""
                xs = wk.tile([128, EK, 4 * BL], bf, tag="xs", name="xs")
                for e in range(EK):
                    nc.gpsimd.tensor_copy(xs[:, e, :],
                                          xT_sb[:, e, bass.ds(t0 * BL, 4 * BL)])
                return xs

            def xp_mms(blk, xs, s, gate=None):
                """Emit this step's share of the next block's xp matmuls.

                gate: a PE instruction of the current step; the xp matmuls'
                input tiles are marked after it so the scheduler cannot
                hoist every step's share into one clump behind step 0
                (greedy idle-fill), which would stall step 1's recurrence
                matmuls behind ~4us of cold xp work. Same-engine gating
                adds no semaphore waits, only queue order."""
                if gate is not None:
                    tc.dep_state.set_after_insts(xs.name, gate.ins)
                    tc.dep_state.set_after_insts(biasx_sb.name, gate.ins)
                if s in XP_BIAS:
                    g = XP_BIAS[s]
                    nc.tensor.matmul(blk[:, g * 512:(g + 1) * 512],
                                     biasx_sb[:, :],
                                     indx_sb[:, g * 512:(g + 1) * 512],
                                     start=True, stop=False)
                for c in XP_CHUNKS[s]:
                    for e in range(EK):
                        nc.tensor.matmul(
                            blk[:, c * 128:(c + 1) * 128],
                            wih_sb[:, e, c * 128:(c + 1) * 128],
                            xs[:, e, :],
                            start=False, stop=(e == EK - 1),
                            skip_group_check=True)

            def nh4_bias(nh4):
                """One indicator matmul fills the block's n-gate bias for all
                4 steps: nh4[p, s, c*BL+b] = bhn[c*128+p]."""
                nc.tensor.matmul(nh4[:, :], bhn_sb[:, :], indh_sb[:, :],
                                 start=True, stop=False)

            def step(blk, nh4, s, h_cur, h_nxt, tail):
                """One GRU step: consumes column s (0..3) of psum block."""
                co = s * BL  # column offset of this step inside the block

                # r/z h-projection accumulates straight onto xp+bias
                for c in range(8):
                    for k in range(HK):
                        nc.tensor.matmul(
                            blk[:, c * 128 + co: c * 128 + co + BL],
                            whh_sb[:, k, c * 128:(c + 1) * 128],
                            h_cur[:, k, :],
                            start=False, stop=(k == HK - 1),
                            skip_group_check=True)

                # fused sigmoid over all r,z chunks -> bf16. Emitted BEFORE
                # the n matmuls: its PE watermark covers only the r/z ones.
                rz = wk.tile([128, 8, BL], bf, tag="rz", name="rz")
                nc.scalar.activation(
                    rz[:, :, :],
                    blk[:, :].rearrange("p (c n) -> p c n", c=NG)[:, 0:8, co:co + BL],
                    ACT.Sigmoid)
                # off-chain: z = 1 - z'; b2 = z*h   (ready well before h')
                # (GPSIMD/Pool has no PSUM access: all PSUM readers go to
                # DVE or ACT; all-SBUF bf16 ops go to Pool/DVE-4x.)
                zf = wk.tile([128, HK, BL], bf, tag="zf", name="zf")
                nc.gpsimd.tensor_scalar(zf[:, :, :], rz[:, 4:8, :], -1.0, 1.0,
                                        ALU.mult, ALU.add)
                b2 = wk.tile([128, HK, BL], bf, tag="b2", name="b2")
                nc.gpsimd.tensor_mul(b2[:, :, :], zf[:, :, :], h_cur[:, :, :])

                # n-gate h-projection accumulates onto the block nh4 bias
                last_nh = None
                for c in range(HK):
                    for k in range(HK):
                        last_nh = nc.tensor.matmul(
                            nh4[:, s, c * BL:(c + 1) * BL],
                            whh_sb[:, k, (8 + c) * 128:(9 + c) * 128],
                            h_cur[:, k, :],
                            start=False, stop=(k == HK - 1),
                            skip_group_check=True)

                # n chain: t2 = r * (hn + bhn); v = t2 + xp_n; n = tanh(v)
                t2 = wk.tile([128, HK, BL], bf, tag="t2", name="t2")
                nc.vector.tensor_mul(
                    t2[:, :, :], rz[:, 0:4, :],
                    nh4[:, s, :].rearrange("p (c n) -> p c n", c=HK))
                v = wk.tile([128, HK, BL], bf, tag="v", name="v")
                nc.vector.tensor_add(
                    v[:, :, :], t2[:, :, :],
                    blk[:, :].rearrange("p (c n) -> p c n", c=NG)[:, 8:12, co:co + BL])
                n_t = wk.tile([128, HK, BL], bf, tag="n", name="n")
                nc.scalar.activation(n_t[:, :, :], v[:, :, :], ACT.Tanh)
                # h' = z'*n + z*h
                a_t = wk.tile([128, HK, BL], bf, tag="a", name="a")
                nc.vector.tensor_mul(a_t[:, :, :], rz[:, 4:8, :], n_t[:, :, :])
                nc.vector.tensor_add(h_nxt[:, :, :], a_t[:, :, :], b2[:, :, :])

                # next block's xp share fills the PE tail of this step
                tail(s, last_nh)

            # prologue: produce block 0 (steps 0..3) and its nh4 bias
            xs0 = xp_stage(0)
            for s in range(4):
                xp_mms(blk_bufs[0], xs0, s)
            nh4_bias(nh_bufs[0][:, :, :].rearrange("p c n -> p (c n)"))

            ASSUMED_EVEN_BLOCKS = (U // 4) % 2 == 0
            assert ASSUMED_EVEN_BLOCKS and U % 4 == 0

            with tc.For_i(0, T // U, 1, staggered_reset=True,
                          hint_engines=(mybir.EngineType.PE,)) as it:
                t_base = it * U
                for u4 in range(U // 4):
                    blk_cur, nh4_cur = blk_bufs[u4 % 2], nh_bufs[u4 % 2]
                    blk_nxt, nh4_nxt = blk_bufs[(u4 + 1) % 2], nh_bufs[(u4 + 1) % 2]
                    xs_nxt = xp_stage(t_base + u4 * 4 + 4)

                    def tail(s, gate, b=blk_nxt, x=xs_nxt, n4=nh4_nxt):
                        xp_mms(b, x, s, gate)
                        if s == 3:
                            tc.dep_state.set_after_insts(bhn_sb.name, gate.ins)
                            nh4_bias(n4[:, :, :].rearrange("p c n -> p (c n)"))

                    for s in range(4):
                        t = u4 * 4 + s          # static step index in body
                        step(blk_cur, nh4_cur, s,
                             h_sb[t % 4], h_sb[(t + 1) % 4], tail)

            # ---- fc partial: y[b] = sum_k fcw[:,k] . h[:,k,b] ----
            fc_ps = ps.tile([1, BL], f32, tag="nh", name="fc")
            for k in range(HK):
                nc.tensor.matmul(fc_ps[:, :], fcw_sb[:, k:k + 1],
                                 h_sb[T % 4][:, k, :],
                                 start=(k == 0), stop=(k == HK - 1))
            y_sb = cst.tile([1, BL], f32, tag="y")
            nc.scalar.copy(y_sb[:, :], fc_ps[:, :])
            nc.sync.dma_start(yout[:, :], y_sb[:, :])
    nc.finalize()
    return nc


_NC_CACHE = None


def _get_nc():
    global _NC_CACHE
    if _NC_CACHE is None:
        _NC_CACHE = _build_nc()
    return _NC_CACHE


def _prep_xT(x_c, rev):
    """x_c [BL, T, E] f32 -> [EK, 128, TP*BL] bf16 (optionally reversed)."""
    if rev:
        x_c = x_c[:, ::-1, :]
    xt = np.zeros((EK, 128, TP * BL), np.float32)
    # xt[e, p, t*BL + b] = x_c[b, t, 128e + p]
    xt[:, :, :T * BL] = np.ascontiguousarray(
        x_c.transpose(2, 1, 0)).reshape(EK, 128, T * BL)
    return xt.astype(BF)


def _prep_weights(W_ih, W_hh, b_ih, b_hh):
    Wi = np.array(W_ih, np.float32).copy()
    Wh = np.array(W_hh, np.float32).copy()
    Wi[H:2 * H] *= -1.0
    Wh[H:2 * H] *= -1.0
    # whh[k] = Whh'[:, 128k:128k+128].T  -> [128, G]
    whhT = np.ascontiguousarray(Wh.T).reshape(HK, 128, G).astype(BF)
    wihT = np.ascontiguousarray(Wi.T).reshape(EK, 128, G).astype(BF)
    bi = np.asarray(b_ih, np.float32)
    bh = np.asarray(b_hh, np.float32)
    vb = np.concatenate([
        bi[0:H] + bh[0:H],
        -(bi[H:2 * H] + bh[H:2 * H]),
        bi[2 * H:3 * H]])
    biasx = vb.reshape(NG, 128).astype(BF)
    bhn_ = bh[2 * H:3 * H].reshape(HK, 128).astype(BF)
    return wihT, whhT, biasx, bhn_


def prepare_in_maps(inputs, emb, W_ih_f, W_hh_f, b_ih_f, b_hh_f,
                    W_ih_b, W_hh_b, b_ih_b, b_hh_b, fc_w, fc_b):
    ids = np.asarray(inputs)
    emb = np.asarray(emb, np.float32)
    x = emb[ids]  # [B, T, E]

    indx_ = np.zeros((NG, NG * 128), np.float32)
    for k in range(NG):
        indx_[k, k * 128:(k + 1) * 128] = 1.0
    indh1 = np.zeros((HK, HK * BL), np.float32)
    for k in range(HK):
        indh1[k, k * BL:(k + 1) * BL] = 1.0
    indh_ = np.tile(indh1, (1, 4))          # [HK, 4*HK*BL]
    fc = np.asarray(fc_w, np.float32)[0]
    ones = np.ones((1, BL), np.float32)

    per_dir = {}
    for d, (Wi, Wh, bi, bh) in (
            ("f", (W_ih_f, W_hh_f, b_ih_f, b_hh_f)),
            ("b", (W_ih_b, W_hh_b, b_ih_b, b_hh_b))):
        wihT, whhT, biasx, bhn_ = _prep_weights(Wi, Wh, bi, bh)
        off = 0 if d == "f" else H
        fcw_ = fc[off:off + H].reshape(HK, 128).T.copy().astype(BF)  # [128, HK]
        per_dir[d] = dict(whh=whhT, wih=wihT, biasx=biasx, bhn=bhn_,
                          fcw=fcw_, indx=indx_.astype(BF),
                          indh=indh_.astype(BF), ones=ones.astype(BF))

    in_maps = []
    for c in range(NC):
        d = "f" if c < 4 else "b"
        sh = c % 4
        x_c = x[sh * BL:(sh + 1) * BL]
        in_maps.append(dict(per_dir[d], xT=_prep_xT(x_c, d == "b")))
    return in_maps


def kernel(**inputs):
    in_maps = prepare_in_maps(**inputs)
    nc = _get_nc()
    res = run_bass_kernel_spmd(nc, in_maps, core_ids=list(range(NC)))
    fcb = np.float32(np.asarray(inputs["fc_b"], np.float32).reshape(-1)[0])
    out = np.empty((B, 1), np.float32)
    for sh in range(4):
        yf = np.asarray(res.results[sh]["y"], np.float32).reshape(BL)
        yb = np.asarray(res.results[4 + sh]["y"], np.float32).reshape(BL)
        out[sh * BL:(sh + 1) * BL, 0] = 1.0 / (1.0 + np.exp(-(yf + yb + fcb)))
    return out
